# revision 46
# baseline (speedup 1.0000x reference)
"""Trainium2 Bass kernel for single-head causal attention (B=4, T=2048, C=2048).

Sharding: 8 cores = 4 batches x 2 t-shards. Core (b, h) owns 256-row t-blocks
{0,3,4,7} (h=0) / {1,2,5,6} (h=1) of batch b, mapped onto 4 uniform slots with
static key extents [4,8,12,16]x128 so a single SPMD program serves both shards
(slot overshoot is fully masked). Each core computes K^T / V for its s-half and
the pair exchanges them via 2-core AllGathers, split into d-quarters (K) and
s-halves (V) issued inside the producing loops so transfer and the SBUF
gather-loads overlap the Q projection instead of gating the attention phases.

All matmuls run in bf16 (fp32r self-loads its weights serially every matmul;
bf16 keeps the same 1 cycle/row stream rate without that), accumulating fp32
in PSUM. K^T, V, Q^T and the attention output stay SBUF-resident; adjacent
slots are processed as pairs so scores/AV stream N=512 in the shared causal
range. Softmax runs unnormalized in the transposed [s,t] domain; denominators
come from a ones-matmul partition reduction and scale the final projection.
"""
import sys

sys.path.insert(0, "/opt/trn_rl_repo")
import numpy as np
import ml_dtypes

_CACHE = {}

B = 4
T_FULL = 2048
C_FULL = 2048
NEG = -1e30
OWN = {0: [0, 3, 4, 7], 1: [1, 2, 5, 6]}
BF16NP = ml_dtypes.bfloat16


def _build(T_, C_, reps=1, skip_cc=False, rep_isolated=False, lesion=None):
    import concourse.bacc as bacc
    import concourse.mybir as mybir
    import concourse.tile as tile

    F32 = mybir.dt.float32
    BF16 = mybir.dt.bfloat16
    AF = mybir.ActivationFunctionType
    SCALE = 1.0 / float(np.sqrt(C_FULL))

    CC = C_ // 128      # contraction 128-chunks (also d-chunks)
    TOWN = T_ // 2      # owned t rows (and s-half size)
    NSL = TOWN // 128   # s-chunks per half
    NS = T_ // 128      # total s-chunks
    NBO = TOWN // 256   # owned 256-blocks (slots)
    NE = C_ // 512      # 512-wide d/e blocks
    NQ = TOWN // 512
    ext = [4 * (bi + 1) for bi in range(NBO)]

    nc = bacc.Bacc("TRN2", target_bir_lowering=False, debug=False, num_devices=8)
    xth_d = nc.declare_dram_parameter("xth", [128, CC, TOWN], BF16, isOutput=False)
    xtq_d = nc.declare_dram_parameter("xtq", [128, CC, TOWN], BF16, isOutput=False)
    wq_d = nc.declare_dram_parameter("wq", [CC, 128, CC, 128], BF16, isOutput=False)
    wk_d = nc.declare_dram_parameter("wk", [CC, 128, CC, 128], BF16, isOutput=False)
    wv_d = nc.declare_dram_parameter("wv", [NE, 128, CC, 512], BF16, isOutput=False)
    wo_d = nc.declare_dram_parameter("wo", [NE, 128, CC, 512], BF16, isOutput=False)
    mb_d = nc.declare_dram_parameter("mb", [NBO, 4, 128, 256], BF16, isOutput=False)
    ones_d = nc.declare_dram_parameter("ones", [128, 2], BF16, isOutput=False)
    y_d = nc.declare_dram_parameter("y", [TOWN, C_], F32, isOutput=True)

    groups = [[0, 1], [2, 3], [4, 5], [6, 7]]
    n_wide = sum(min(e, ext[2 * i]) for i, e in enumerate(ext[1::2]))
    n_narrow = sum(e - ext[2 * i] for i, e in enumerate(ext[1::2]))

    with tile.TileContext(nc) as tc:
        with tc.tile_pool(name="dram", bufs=1, space="DRAM") as dram:
            NKQ = CC // 4      # K AllGather d-quarters
            NVH = TOWN // 512  # V AllGather s-halves
            scratch = []
            for i in range(reps if rep_isolated else 1):
                scratch.append((
                    dram.tile([1, C_, TOWN], BF16, tag=f"kth{i}", name=f"kth{i}"),
                    dram.tile([NKQ, 2, C_ // NKQ, TOWN], BF16, tag=f"kts{i}", name=f"kts{i}"),
                    dram.tile([1, TOWN, C_], BF16, tag=f"vh{i}", name=f"vh{i}"),
                    dram.tile([NVH, 2, 512, C_], BF16, tag=f"vs{i}", name=f"vs{i}"),
                ))
            with (
                tc.tile_pool(name="stage", bufs=3) as stage,
                tc.tile_pool(name="cst", bufs=1) as cst,
            ):
                onest = cst.tile([128, 2], BF16, tag="ones")
                nc.sync.dma_start(onest[:], ones_d[:])
                recipt = cst.tile([128, 2 * NBO], F32, tag="recip")
                for _rep in range(reps):
                    KT_h, KT_s, V_h, V_s = scratch[_rep if rep_isolated else 0]
                    # ==== pools alive from P1 through attention ====
                    with (
                        tc.tile_pool(name="mbp", bufs=1) as pool_mb,
                        tc.tile_pool(name="vp", bufs=1) as pool_v,
                    ):
                        mbt = pool_mb.tile([128, NBO, 4, 256], BF16, tag="mb")
                        nc.sync.dma_start(mbt[:], mb_d[:].rearrange("s r p n -> p s r n"))
                        v = pool_v.tile([128, NS, C_], BF16, tag="v")
                        with (
                            tc.tile_pool(name="xt", bufs=1) as pool_xt,
                            tc.tile_pool(name="wvp", bufs=1) as pool_wv,
                        ):
                            xt = pool_xt.tile([128, CC, TOWN], BF16, tag="xt")
                            for c in range(CC):
                                nc.sync.dma_start(xt[:, c, :], xth_d[:, c, :])
                            # wv preloads flow during P1a (pool hoisted above the
                            # wk scope so its first DMA has no SBUF WAR on P1a)
                            wv = pool_wv.tile([128, CC, C_], BF16, tag="wv")
                            for dd in range(NE):
                                nc.sync.dma_start(wv[:, :, 512 * dd:512 * dd + 512], wv_d[dd])
                            # ==== P1a: K^T(own half) = Wk^T.T @ xT ====
                            with (
                                tc.tile_pool(name="wkp", bufs=2) as pool_wk,
                                tc.tile_pool(name="psk", bufs=6, space="PSUM") as psk,
                            ):
                                for d in range(CC):
                                    wk = pool_wk.tile([128, CC, 128], BF16, tag="wk")
                                    nc.sync.dma_start(wk[:], wk_d[d])
                                    kps = [
                                        psk.tile([128, 512], F32, tag="kps", name=f"kps{d}_{ss}")
                                        for ss in range(NQ)
                                    ]
                                    for c in range(CC):
                                        for ss in range(NQ):
                                            nc.tensor.matmul(
                                                kps[ss][:],
                                                wk[:, c, :],
                                                xt[:, c, 512 * ss:512 * ss + 512],
                                                start=(c == 0),
                                                stop=(c == CC - 1),
                                            )
                                    for ss in range(NQ):
                                        st = stage.tile([128, 512], BF16, tag="st512")
                                        nc.vector.tensor_copy(st[:], kps[ss][:])
                                        nc.sync.dma_start(
                                            KT_h[0, 128 * d:128 * d + 128, 512 * ss:512 * ss + 512],
                                            st[:],
                                        )
                                    # gather each finished K^T d-quarter right away
                                    if not skip_cc and (d + 1) % 4 == 0:
                                        q = d // 4
                                        nc.gpsimd.collective_compute(
                                            "AllGather",
                                            mybir.AluOpType.bypass,
                                            replica_groups=groups,
                                            ins=[KT_h[0, 512 * q:512 * q + 512, :]],
                                            outs=[KT_s[q]],
                                        )
                            # ==== P1b: V(own half) = xT.T @ Wv^T; V halves are
                            # gathered and loaded to SBUF as they finish ====
                            with tc.tile_pool(name="psv", bufs=8, space="PSUM") as psv:
                                for sl in range(NSL):
                                    vps = [
                                        psv.tile([128, 512], F32, tag="vps", name=f"vps{sl}_{dd}")
                                        for dd in range(NE)
                                    ]
                                    for c in range(CC):
                                        for dd in range(NE):
                                            nc.tensor.matmul(
                                                vps[dd][:],
                                                xt[:, c, 128 * sl:128 * sl + 128],
                                                wv[:, c, 512 * dd:512 * dd + 512],
                                                start=(c == 0),
                                                stop=(c == CC - 1),
                                            )
                                    for dd in range(NE):
                                        st = stage.tile([128, 512], BF16, tag="st512")
                                        nc.vector.tensor_copy(st[:], vps[dd][:])
                                        nc.sync.dma_start(
                                            V_h[0, 128 * sl:128 * sl + 128, 512 * dd:512 * dd + 512],
                                            st[:],
                                        )
                                    if not skip_cc and (sl + 1) % 4 == 0:
                                        hq = sl // 4
                                        nc.gpsimd.collective_compute(
                                            "AllGather",
                                            mybir.AluOpType.bypass,
                                            replica_groups=groups,
                                            ins=[V_h[0, 512 * hq:512 * hq + 512, :]],
                                            outs=[V_s[hq]],
                                        )
                        # v gather-loads flow during P1c on the gpsimd queue
                        # (right behind the AllGathers that produce V_s)
                        if lesion != "noload":
                            for hq in range(NVH):
                                for half in range(2):
                                    for sq in range(4):
                                        nc.gpsimd.dma_start(
                                            v[:, NSL * half + 4 * hq + sq, :],
                                            V_s[hq, half, 128 * sq:128 * sq + 128, :],
                                        )
                        # ==== attention: scores -> denominators -> AV -> P3 ====
                        with tc.tile_pool(name="attnp", bufs=1) as pool_attn:
                            attn = {}
                            with (
                                tc.tile_pool(name="ktap", bufs=1) as pool_kta,
                                tc.tile_pool(name="qtp", bufs=1) as pool_qt,
                            ):
                                # K^T s-half 0 loads overlap P1c; half 1 lands
                                # during early scores (scores consume s ascending)
                                kta = pool_kta.tile([128, CC, TOWN], BF16, tag="kta")
                                if lesion != "noload":
                                    for dc in range(CC):
                                        nc.scalar.dma_start(
                                            kta[:, dc, :],
                                            KT_s[dc // 4, 0, 128 * (dc % 4):128 * (dc % 4) + 128, :],
                                        )
                                qt = pool_qt.tile([128, CC, TOWN], BF16, tag="qt")
                                # ==== P1c: Q^T = Wq^T.T @ xTq (AGs/loads overlap) ====
                                with (
                                    tc.tile_pool(name="xtqp", bufs=1) as pool_xtq,
                                    tc.tile_pool(name="wqp", bufs=2) as pool_wq,
                                    tc.tile_pool(name="psq", bufs=6, space="PSUM") as psq,
                                ):
                                    xtq = pool_xtq.tile([128, CC, TOWN], BF16, tag="xtq")
                                    for c in range(CC):
                                        nc.sync.dma_start(xtq[:, c, :], xtq_d[:, c, :])
                                    for d in range(CC):
                                        wq = pool_wq.tile([128, CC, 128], BF16, tag="wq")
                                        nc.sync.dma_start(wq[:], wq_d[d])
                                        qps = [
                                            psq.tile([128, 512], F32, tag="qps", name=f"qps{d}_{tt}")
                                            for tt in range(NQ)
                                        ]
                                        for c in range(CC):
                                            for tt in range(NQ):
                                                nc.tensor.matmul(
                                                    qps[tt][:],
                                                    wq[:, c, :],
                                                    xtq[:, c, 512 * tt:512 * tt + 512],
                                                    start=(c == 0),
                                                    stop=(c == CC - 1),
                                                )
                                        for tt in range(NQ):
                                            nc.vector.tensor_copy(
                                                qt[:, d, 512 * tt:512 * tt + 512], qps[tt][:]
                                            )
                                # ==== P2a: scores + exp, slot-paired ====
                                with (
                                    tc.tile_pool(name="ktbp", bufs=1) as pool_ktb,
                                    tc.tile_pool(name="pssc", bufs=6, space="PSUM") as pssc,
                                    tc.tile_pool(name="psscn", bufs=2, space="PSUM") as psscn,
                                ):
                                    ktb = pool_ktb.tile([128, CC, TOWN], BF16, tag="ktb")
                                    if lesion == "noload":
                                        nc.vector.memset(kta[:], 0.0)
                                        nc.vector.memset(ktb[:], 0.0)
                                        nc.vector.memset(v[:], 0.0)
                                    else:
                                        for dc in range(CC):
                                            nc.scalar.dma_start(
                                                ktb[:, dc, :],
                                                KT_s[dc // 4, 1, 128 * (dc % 4):128 * (dc % 4) + 128, :],
                                            )
                                    for pr in range(NBO // 2 if lesion != "proj" else 0):
                                        lo, hi = 2 * pr, 2 * pr + 1
                                        elo, ehi = ext[lo], ext[hi]
                                        for kl in range(ehi):
                                            wide = kl < elo
                                            if wide:
                                                sps = pssc.tile(
                                                    [128, 512], F32, tag="spsw",
                                                    name=f"sps{pr}_{kl}",
                                                )
                                            else:
                                                sps = psscn.tile(
                                                    [128, 256], F32, tag="spsn",
                                                    name=f"sps{pr}_{kl}",
                                                )
                                            ksrc = kta if kl < NSL else ktb
                                            koff = 128 * (kl % NSL)
                                            for dc in range(CC):
                                                nc.tensor.matmul(
                                                    sps[:],
                                                    ksrc[:, dc, koff:koff + 128],
                                                    qt[:, dc, 512 * pr:512 * pr + 512] if wide
                                                    else qt[:, dc, 512 * pr + 256:512 * pr + 512],
                                                    start=(dc == 0),
                                                    stop=(dc == CC - 1),
                                                )
                                            if wide and kl >= elo - 4:
                                                nc.vector.tensor_add(
                                                    sps[:, 0:256], sps[:, 0:256],
                                                    mbt[:, lo, kl - (elo - 4), :],
                                                )
                                            if not wide and kl >= ehi - 4:
                                                nc.vector.tensor_add(
                                                    sps[:], sps[:],
                                                    mbt[:, hi, kl - (ehi - 4), :],
                                                )
                                            at = pool_attn.tile(
                                                [128, 512] if wide else [128, 256],
                                                BF16,
                                                tag="attnw" if wide else "attnn",
                                                name=f"attn{pr}_{kl}",
                                                bufs=n_wide if wide else n_narrow,
                                            )
                                            nc.scalar.activation(at[:], sps[:], AF.Exp, scale=SCALE)
                                            attn[(pr, kl)] = at
                                # ==== P2b: softmax denominators ====
                                with tc.tile_pool(name="psr", bufs=2, space="PSUM") as psr:
                                    for pr in range(NBO // 2 if lesion != "proj" else 0):
                                        lo, hi = 2 * pr, 2 * pr + 1
                                        for bi in (lo, hi):
                                            for sub in range(2):
                                                rps = psr.tile(
                                                    [128, 2], F32, tag="rps",
                                                    name=f"rps{bi}_{sub}",
                                                )
                                                for kl in range(ext[bi]):
                                                    col = 128 * sub
                                                    if bi == hi and kl < ext[lo]:
                                                        col += 256
                                                    nc.tensor.matmul(
                                                        rps[:],
                                                        attn[(pr, kl)][:, col:col + 128],
                                                        onest[:],
                                                        start=(kl == 0),
                                                        stop=(kl == ext[bi] - 1),
                                                    )
                                                nc.vector.reciprocal(
                                                    recipt[:, 2 * bi + sub:2 * bi + sub + 1],
                                                    rps[:, 0:1],
                                                )
                            # ==== P2c: AV, slot-paired; kl==0 full-width matmul
                            # zero-starts each PSUM bank ====
                            with (
                                tc.tile_pool(name="otp", bufs=1) as pool_ot,
                                tc.tile_pool(name="wop", bufs=1) as pool_wo,
                            ):
                                ot = pool_ot.tile([128, CC, TOWN], BF16, tag="ot")
                                # Wo preloads flow during AV so P3 starts clean
                                wo = pool_wo.tile([128, CC, C_], BF16, tag="wo")
                                for e in range(NE):
                                    nc.sync.dma_start(wo[:, :, 512 * e:512 * e + 512], wo_d[e])
                                with tc.tile_pool(name="psav", bufs=8, space="PSUM") as psav:
                                    if lesion == "proj":
                                        nc.vector.memset(ot[:], 0.0)
                                        nc.vector.memset(recipt[:], 1.0)
                                    for pr in range(NBO // 2 if lesion != "proj" else 0):
                                        lo, hi = 2 * pr, 2 * pr + 1
                                        elo, ehi = ext[lo], ext[hi]
                                        for q in range(CC // 4):
                                            avs = [
                                                psav.tile(
                                                    [128, 512], F32, tag="av",
                                                    name=f"av{pr}_{q}_{dq}",
                                                )
                                                for dq in range(4)
                                            ]
                                            for kl in range(ehi):
                                                wide = kl < elo
                                                at = attn[(pr, kl)]
                                                for dq in range(4):
                                                    dc = 4 * q + dq
                                                    nc.tensor.matmul(
                                                        avs[dq][:] if wide
                                                        else avs[dq][:, 256:512],
                                                        v[:, kl, 128 * dc:128 * dc + 128],
                                                        at[:],
                                                        start=(kl == 0),
                                                        stop=(kl == ehi - 1),
                                                        skip_group_check=True,
                                                    )
                                            for dq in range(4):
                                                dc = 4 * q + dq
                                                nc.vector.tensor_copy(
                                                    ot[:, dc, 512 * pr:512 * pr + 512],
                                                    avs[dq][:],
                                                )
                                # ==== P3: y = (O^T.T @ Wo^T) * recip ====
                                with tc.tile_pool(name="psf", bufs=6, space="PSUM") as psf:
                                    for e in range(NE):
                                        for tsub in range(2 * NBO):
                                            fps = psf.tile(
                                                [128, 512], F32, tag="fps",
                                                name=f"fps{e}_{tsub}",
                                            )
                                            for dc in range(CC):
                                                nc.tensor.matmul(
                                                    fps[:],
                                                    ot[:, dc, 128 * tsub:128 * tsub + 128],
                                                    wo[:, dc, 512 * e:512 * e + 512],
                                                    start=(dc == 0),
                                                    stop=(dc == CC - 1),
                                                )
                                            yt = stage.tile([128, 512], F32, tag="yt")
                                            nc.vector.tensor_scalar_mul(
                                                yt[:], fps[:], recipt[:, tsub:tsub + 1]
                                            )
                                            nc.sync.dma_start(
                                                y_d[128 * tsub:128 * tsub + 128, 512 * e:512 * e + 512],
                                                yt[:],
                                            )
    nc.compile()
    return nc


def _tile_w128(WT, CC):
    # [c, d] -> [dchunk, p, cc, 128] so each per-d-chunk DMA is contiguous
    return np.ascontiguousarray(
        WT.reshape(CC, 128, CC, 128).transpose(2, 1, 0, 3)
    )


def _tile_w512(WT, CC, NE):
    return np.ascontiguousarray(
        WT.reshape(CC, 128, NE, 512).transpose(2, 1, 0, 3)
    )


def _host_prep(x, Wq, Wk, Wv, Wo, T_, C_):
    CC = C_ // 128
    NE = C_ // 512
    TOWN = T_ // 2
    NBO = TOWN // 256
    ext = [4 * (bi + 1) for bi in range(NBO)]
    own_map = {0: OWN[0][:NBO], 1: OWN[1][:NBO]}
    x = np.asarray(x, np.float32)
    WqT = _tile_w128(np.asarray(Wq, np.float32).T.astype(BF16NP), CC)
    WkT = _tile_w128(np.asarray(Wk, np.float32).T.astype(BF16NP), CC)
    WvT = _tile_w512(np.asarray(Wv, np.float32).T.astype(BF16NP), CC, NE)
    WoT = _tile_w512(np.asarray(Wo, np.float32).T.astype(BF16NP), CC, NE)
    ones = np.ones((128, 2), BF16NP)
    masks = {}
    own_cols = {}
    for h in range(2):
        own = own_map[h]
        mb = np.zeros((NBO, 4, 128, 256), np.float32)
        for bi in range(NBO):
            g = own[bi]
            for rel in range(4):
                kl = ext[bi] - 4 + rel
                s_idx = 128 * kl + np.arange(128)[:, None]
                t_idx = 256 * g + np.arange(256)[None, :]
                mb[bi, rel] = np.where(s_idx <= t_idx, 0.0, NEG)
        masks[h] = mb.astype(BF16NP)
        own_cols[h] = np.concatenate(
            [np.arange(256 * g, 256 * g + 256) for g in own]
        )
    in_maps = []
    for core in range(8):
        b, h = core // 2, core % 2
        xb = x[b % x.shape[0]]
        xT = np.ascontiguousarray(xb.T).astype(BF16NP)
        xth = np.ascontiguousarray(
            xT[:, h * TOWN:(h + 1) * TOWN].reshape(CC, 128, TOWN).transpose(1, 0, 2)
        )
        xtq = np.ascontiguousarray(
            xT[:, own_cols[h]].reshape(CC, 128, TOWN).transpose(1, 0, 2)
        )
        in_maps.append(
            {
                "xth": xth,
                "xtq": xtq,
                "wq": WqT,
                "wk": WkT,
                "wv": WvT,
                "wo": WoT,
                "mb": masks[h],
                "ones": ones,
            }
        )
    return in_maps, own_cols


def kernel(x, Wq, Wk, Wv, Wo):
    from concourse.bass_utils import run_bass_kernel_spmd

    T_, C_ = T_FULL, C_FULL
    key = (T_, C_)
    if key not in _CACHE:
        _CACHE[key] = _build(T_, C_)
    nc = _CACHE[key]
    in_maps, own_cols = _host_prep(x, Wq, Wk, Wv, Wo, T_, C_)
    res = run_bass_kernel_spmd(nc, in_maps, list(range(8)))
    NBO = (T_ // 2) // 256
    y = np.zeros((B, T_, C_), np.float32)
    for core in range(8):
        b, h = core // 2, core % 2
        yc = res.results[core]["y"]
        for bi in range(NBO):
            g = OWN[h][:NBO][bi]
            y[b, 256 * g:256 * g + 256, :] = yc[256 * bi:256 * bi + 256, :]
    return y


# revision 51
# speedup vs baseline: 1.2345x; 1.2345x over previous
"""Trainium2 Bass kernel for single-head causal attention (B=4, T=2048, C=2048).

Sharding: 8 cores = 4 batches x 2 t-shards. Core (b, h) owns 256-row t-blocks
{0,3,4,7} (h=0) / {1,2,5,6} (h=1) of batch b, mapped onto 4 uniform slots with
static key extents [4,8,12,16]x128 so a single SPMD program serves both shards
(slot overshoot is fully masked). Each core computes K^T / V for its s-half and
the pair exchanges them via 2-core AllGathers, split into d-quarters (K) and
s-halves (V) issued inside the producing loops so transfer and the SBUF
gather-loads overlap the Q projection instead of gating the attention phases.

All matmuls run in bf16 (fp32r self-loads its weights serially every matmul;
bf16 keeps the same 1 cycle/row stream rate without that), accumulating fp32
in PSUM. K^T, V, Q^T and the attention output stay SBUF-resident; adjacent
slots are processed as pairs so scores/AV stream N=512 in the shared causal
range. Softmax runs unnormalized in the transposed [s,t] domain; denominators
come from a ones-matmul partition reduction and scale the final projection.
"""
import sys

sys.path.insert(0, "/opt/trn_rl_repo")
import numpy as np
import ml_dtypes

_CACHE = {}

B = 4
T_FULL = 2048
C_FULL = 2048
NEG = -1e30
OWN = {0: [0, 3, 4, 7], 1: [1, 2, 5, 6]}
BF16NP = ml_dtypes.bfloat16


def _build(T_, C_, reps=1, skip_cc=False, rep_isolated=False, lesion=None):
    import concourse.bacc as bacc
    import concourse.mybir as mybir
    import concourse.tile as tile

    F32 = mybir.dt.float32
    BF16 = mybir.dt.bfloat16
    AF = mybir.ActivationFunctionType
    SCALE = 1.0 / float(np.sqrt(C_FULL))

    CC = C_ // 128      # contraction 128-chunks (also d-chunks)
    TOWN = T_ // 2      # owned t rows (and s-half size)
    NSL = TOWN // 128   # s-chunks per half
    NS = T_ // 128      # total s-chunks
    NBO = TOWN // 256   # owned 256-blocks (slots)
    NE = C_ // 512      # 512-wide d/e blocks
    NQ = TOWN // 512
    ext = [4 * (bi + 1) for bi in range(NBO)]

    nc = bacc.Bacc("TRN2", target_bir_lowering=False, debug=False, num_devices=8)
    xth_d = nc.declare_dram_parameter("xth", [128, CC, TOWN], BF16, isOutput=False)
    xtq_d = nc.declare_dram_parameter("xtq", [128, CC, TOWN], BF16, isOutput=False)
    wq_d = nc.declare_dram_parameter("wq", [CC, 128, CC, 128], BF16, isOutput=False)
    wk_d = nc.declare_dram_parameter("wk", [CC, 128, CC, 128], BF16, isOutput=False)
    wv_d = nc.declare_dram_parameter("wv", [NE, 128, CC, 512], BF16, isOutput=False)
    wo_d = nc.declare_dram_parameter("wo", [NE, 128, CC, 512], BF16, isOutput=False)
    mb_d = nc.declare_dram_parameter("mb", [NBO, 4, 128, 256], BF16, isOutput=False)
    ones_d = nc.declare_dram_parameter("ones", [128, 2], BF16, isOutput=False)
    y_d = nc.declare_dram_parameter("y", [TOWN, C_], F32, isOutput=True)

    groups = [[0, 1], [2, 3], [4, 5], [6, 7]]
    n_wide = sum(min(e, ext[2 * i]) for i, e in enumerate(ext[1::2]))
    n_narrow = sum(e - ext[2 * i] for i, e in enumerate(ext[1::2]))

    with tile.TileContext(nc) as tc:
        with tc.tile_pool(name="dram", bufs=1, space="DRAM") as dram:
            NKQ = CC // 4      # K AllGather d-quarters
            NVH = TOWN // 512  # V AllGather s-halves
            scratch = []
            for i in range(reps if rep_isolated else 1):
                scratch.append((
                    dram.tile([1, C_, TOWN], BF16, tag=f"kth{i}", name=f"kth{i}"),
                    dram.tile([NKQ, 2, C_ // NKQ, TOWN], BF16, tag=f"kts{i}", name=f"kts{i}"),
                    dram.tile([1, TOWN, C_], BF16, tag=f"vh{i}", name=f"vh{i}"),
                    dram.tile([NVH, 2, 512, C_], BF16, tag=f"vs{i}", name=f"vs{i}"),
                ))
            with (
                tc.tile_pool(name="stage", bufs=3) as stage,
                tc.tile_pool(name="cst", bufs=1) as cst,
            ):
                onest = cst.tile([128, 2], BF16, tag="ones")
                nc.sync.dma_start(onest[:], ones_d[:])
                recipt = cst.tile([128, 2 * NBO], F32, tag="recip")
                for _rep in range(reps):
                    KT_h, KT_s, V_h, V_s = scratch[_rep if rep_isolated else 0]
                    # ==== pools alive from P1 through attention ====
                    with (
                        tc.tile_pool(name="mbp", bufs=1) as pool_mb,
                        tc.tile_pool(name="vp", bufs=1) as pool_v,
                    ):
                        mbt = pool_mb.tile([128, NBO, 4, 256], BF16, tag="mb")
                        nc.sync.dma_start(mbt[:], mb_d[:].rearrange("s r p n -> p s r n"))
                        v = pool_v.tile([128, NS, C_], BF16, tag="v")
                        with (
                            tc.tile_pool(name="xt", bufs=1) as pool_xt,
                            tc.tile_pool(name="wvp", bufs=1) as pool_wv,
                        ):
                            xt = pool_xt.tile([128, CC, TOWN], BF16, tag="xt")
                            for c in range(CC):
                                nc.sync.dma_start(xt[:, c, :], xth_d[:, c, :])
                            # wv preloads flow during P1a on the otherwise-idle
                            # scalar queue (pool hoisted above the wk scope so its
                            # first DMA has no SBUF WAR on P1a)
                            wv = pool_wv.tile([128, CC, C_], BF16, tag="wv")
                            for dd in range(NE):
                                nc.scalar.dma_start(wv[:, :, 512 * dd:512 * dd + 512], wv_d[dd])
                            # ==== P1a: K^T(own half) = Wk^T.T @ xT ====
                            with (
                                tc.tile_pool(name="wkp", bufs=2) as pool_wk,
                                tc.tile_pool(name="psk", bufs=6, space="PSUM") as psk,
                            ):
                                for d in range(CC):
                                    wk = pool_wk.tile([128, CC, 128], BF16, tag="wk")
                                    nc.sync.dma_start(wk[:], wk_d[d])
                                    kps = [
                                        psk.tile([128, 512], F32, tag="kps", name=f"kps{d}_{ss}")
                                        for ss in range(NQ)
                                    ]
                                    for c in range(CC):
                                        for ss in range(NQ):
                                            nc.tensor.matmul(
                                                kps[ss][:],
                                                wk[:, c, :],
                                                xt[:, c, 512 * ss:512 * ss + 512],
                                                start=(c == 0),
                                                stop=(c == CC - 1),
                                            )
                                    for ss in range(NQ):
                                        st = stage.tile([128, 512], BF16, tag="st512")
                                        nc.vector.tensor_copy(st[:], kps[ss][:])
                                        # store on the gpsimd queue, directly ahead
                                        # of the AllGather that consumes it
                                        nc.gpsimd.dma_start(
                                            KT_h[0, 128 * d:128 * d + 128, 512 * ss:512 * ss + 512],
                                            st[:],
                                        )
                                    # gather each finished K^T d-quarter right away
                                    if not skip_cc and (d + 1) % 4 == 0:
                                        q = d // 4
                                        nc.gpsimd.collective_compute(
                                            "AllGather",
                                            mybir.AluOpType.bypass,
                                            replica_groups=groups,
                                            ins=[KT_h[0, 512 * q:512 * q + 512, :]],
                                            outs=[KT_s[q]],
                                        )
                            # ==== P1b: V(own half) = xT.T @ Wv^T; V halves are
                            # gathered and loaded to SBUF as they finish ====
                            with tc.tile_pool(name="psv", bufs=8, space="PSUM") as psv:
                                for sl in range(NSL):
                                    vps = [
                                        psv.tile([128, 512], F32, tag="vps", name=f"vps{sl}_{dd}")
                                        for dd in range(NE)
                                    ]
                                    for c in range(CC):
                                        for dd in range(NE):
                                            nc.tensor.matmul(
                                                vps[dd][:],
                                                xt[:, c, 128 * sl:128 * sl + 128],
                                                wv[:, c, 512 * dd:512 * dd + 512],
                                                start=(c == 0),
                                                stop=(c == CC - 1),
                                            )
                                    for dd in range(NE):
                                        st = stage.tile([128, 512], BF16, tag="st512")
                                        nc.vector.tensor_copy(st[:], vps[dd][:])
                                        nc.gpsimd.dma_start(
                                            V_h[0, 128 * sl:128 * sl + 128, 512 * dd:512 * dd + 512],
                                            st[:],
                                        )
                                    if not skip_cc and (sl + 1) % 4 == 0:
                                        hq = sl // 4
                                        nc.gpsimd.collective_compute(
                                            "AllGather",
                                            mybir.AluOpType.bypass,
                                            replica_groups=groups,
                                            ins=[V_h[0, 512 * hq:512 * hq + 512, :]],
                                            outs=[V_s[hq]],
                                        )
                        # v gather-loads flow during P1c on the gpsimd queue
                        # (right behind the AllGathers that produce V_s)
                        if lesion != "noload":
                            for hq in range(NVH):
                                for half in range(2):
                                    for sq in range(4):
                                        nc.gpsimd.dma_start(
                                            v[:, NSL * half + 4 * hq + sq, :],
                                            V_s[hq, half, 128 * sq:128 * sq + 128, :],
                                        )
                        # ==== attention: scores -> denominators -> AV -> P3 ====
                        with tc.tile_pool(name="attnp", bufs=1) as pool_attn:
                            attn = {}
                            with (
                                tc.tile_pool(name="ktap", bufs=1) as pool_kta,
                                tc.tile_pool(name="qtp", bufs=1) as pool_qt,
                            ):
                                # K^T s-half 0 loads overlap P1c; half 1 lands
                                # during early scores (scores consume s ascending)
                                kta = pool_kta.tile([128, CC, TOWN], BF16, tag="kta")
                                if lesion != "noload":
                                    for dc in range(CC):
                                        nc.scalar.dma_start(
                                            kta[:, dc, :],
                                            KT_s[dc // 4, 0, 128 * (dc % 4):128 * (dc % 4) + 128, :],
                                        )
                                qt = pool_qt.tile([128, CC, TOWN], BF16, tag="qt")
                                # ==== P1c: Q^T = Wq^T.T @ xTq (AGs/loads overlap) ====
                                with (
                                    tc.tile_pool(name="xtqp", bufs=1) as pool_xtq,
                                    tc.tile_pool(name="wqp", bufs=2) as pool_wq,
                                    tc.tile_pool(name="psq", bufs=6, space="PSUM") as psq,
                                ):
                                    xtq = pool_xtq.tile([128, CC, TOWN], BF16, tag="xtq")
                                    for c in range(CC):
                                        nc.sync.dma_start(xtq[:, c, :], xtq_d[:, c, :])
                                    for d in range(CC):
                                        wq = pool_wq.tile([128, CC, 128], BF16, tag="wq")
                                        nc.sync.dma_start(wq[:], wq_d[d])
                                        qps = [
                                            psq.tile([128, 512], F32, tag="qps", name=f"qps{d}_{tt}")
                                            for tt in range(NQ)
                                        ]
                                        for c in range(CC):
                                            for tt in range(NQ):
                                                nc.tensor.matmul(
                                                    qps[tt][:],
                                                    wq[:, c, :],
                                                    xtq[:, c, 512 * tt:512 * tt + 512],
                                                    start=(c == 0),
                                                    stop=(c == CC - 1),
                                                )
                                        for tt in range(NQ):
                                            nc.vector.tensor_copy(
                                                qt[:, d, 512 * tt:512 * tt + 512], qps[tt][:]
                                            )
                                # ==== P2a: scores + exp, slot-paired ====
                                with (
                                    tc.tile_pool(name="ktbp", bufs=1) as pool_ktb,
                                    tc.tile_pool(name="pssc", bufs=6, space="PSUM") as pssc,
                                    tc.tile_pool(name="psscn", bufs=2, space="PSUM") as psscn,
                                ):
                                    ktb = pool_ktb.tile([128, CC, TOWN], BF16, tag="ktb")
                                    if lesion == "noload":
                                        nc.vector.memset(kta[:], 0.0)
                                        nc.vector.memset(ktb[:], 0.0)
                                        nc.vector.memset(v[:], 0.0)
                                    else:
                                        for dc in range(CC):
                                            nc.scalar.dma_start(
                                                ktb[:, dc, :],
                                                KT_s[dc // 4, 1, 128 * (dc % 4):128 * (dc % 4) + 128, :],
                                            )
                                    for pr in range(NBO // 2 if lesion != "proj" else 0):
                                        lo, hi = 2 * pr, 2 * pr + 1
                                        elo, ehi = ext[lo], ext[hi]
                                        for kl in range(ehi):
                                            wide = kl < elo
                                            if wide:
                                                sps = pssc.tile(
                                                    [128, 512], F32, tag="spsw",
                                                    name=f"sps{pr}_{kl}",
                                                )
                                            else:
                                                sps = psscn.tile(
                                                    [128, 256], F32, tag="spsn",
                                                    name=f"sps{pr}_{kl}",
                                                )
                                            ksrc = kta if kl < NSL else ktb
                                            koff = 128 * (kl % NSL)
                                            for dc in range(CC):
                                                nc.tensor.matmul(
                                                    sps[:],
                                                    ksrc[:, dc, koff:koff + 128],
                                                    qt[:, dc, 512 * pr:512 * pr + 512] if wide
                                                    else qt[:, dc, 512 * pr + 256:512 * pr + 512],
                                                    start=(dc == 0),
                                                    stop=(dc == CC - 1),
                                                )
                                            if wide and kl >= elo - 4:
                                                nc.vector.tensor_add(
                                                    sps[:, 0:256], sps[:, 0:256],
                                                    mbt[:, lo, kl - (elo - 4), :],
                                                )
                                            if not wide and kl >= ehi - 4:
                                                nc.vector.tensor_add(
                                                    sps[:], sps[:],
                                                    mbt[:, hi, kl - (ehi - 4), :],
                                                )
                                            at = pool_attn.tile(
                                                [128, 512] if wide else [128, 256],
                                                BF16,
                                                tag="attnw" if wide else "attnn",
                                                name=f"attn{pr}_{kl}",
                                                bufs=n_wide if wide else n_narrow,
                                            )
                                            nc.scalar.activation(at[:], sps[:], AF.Exp, scale=SCALE)
                                            attn[(pr, kl)] = at
                                # ==== P2b: softmax denominators ====
                                with tc.tile_pool(name="psr", bufs=2, space="PSUM") as psr:
                                    for pr in range(NBO // 2 if lesion != "proj" else 0):
                                        lo, hi = 2 * pr, 2 * pr + 1
                                        for bi in (lo, hi):
                                            for sub in range(2):
                                                rps = psr.tile(
                                                    [128, 2], F32, tag="rps",
                                                    name=f"rps{bi}_{sub}",
                                                )
                                                for kl in range(ext[bi]):
                                                    col = 128 * sub
                                                    if bi == hi and kl < ext[lo]:
                                                        col += 256
                                                    nc.tensor.matmul(
                                                        rps[:],
                                                        attn[(pr, kl)][:, col:col + 128],
                                                        onest[:],
                                                        start=(kl == 0),
                                                        stop=(kl == ext[bi] - 1),
                                                    )
                                                nc.vector.reciprocal(
                                                    recipt[:, 2 * bi + sub:2 * bi + sub + 1],
                                                    rps[:, 0:1],
                                                )
                            # ==== P2c: AV, slot-paired; kl==0 full-width matmul
                            # zero-starts each PSUM bank ====
                            with (
                                tc.tile_pool(name="otp", bufs=1) as pool_ot,
                                tc.tile_pool(name="wop", bufs=1) as pool_wo,
                            ):
                                ot = pool_ot.tile([128, CC, TOWN], BF16, tag="ot")
                                # Wo preloads flow during AV so P3 starts clean
                                wo = pool_wo.tile([128, CC, C_], BF16, tag="wo")
                                for e in range(NE):
                                    nc.sync.dma_start(wo[:, :, 512 * e:512 * e + 512], wo_d[e])
                                with tc.tile_pool(name="psav", bufs=8, space="PSUM") as psav:
                                    if lesion == "proj":
                                        nc.vector.memset(ot[:], 0.0)
                                        nc.vector.memset(recipt[:], 1.0)
                                    for pr in range(NBO // 2 if lesion != "proj" else 0):
                                        lo, hi = 2 * pr, 2 * pr + 1
                                        elo, ehi = ext[lo], ext[hi]
                                        for q in range(CC // 4):
                                            avs = [
                                                psav.tile(
                                                    [128, 512], F32, tag="av",
                                                    name=f"av{pr}_{q}_{dq}",
                                                )
                                                for dq in range(4)
                                            ]
                                            for kl in range(ehi):
                                                wide = kl < elo
                                                at = attn[(pr, kl)]
                                                for dq in range(4):
                                                    dc = 4 * q + dq
                                                    nc.tensor.matmul(
                                                        avs[dq][:] if wide
                                                        else avs[dq][:, 256:512],
                                                        v[:, kl, 128 * dc:128 * dc + 128],
                                                        at[:],
                                                        start=(kl == 0),
                                                        stop=(kl == ehi - 1),
                                                        skip_group_check=True,
                                                    )
                                            for dq in range(4):
                                                dc = 4 * q + dq
                                                nc.vector.tensor_copy(
                                                    ot[:, dc, 512 * pr:512 * pr + 512],
                                                    avs[dq][:],
                                                )
                                # ==== P3: y = (O^T.T @ Wo^T) * recip ====
                                with tc.tile_pool(name="psf", bufs=6, space="PSUM") as psf:
                                    for e in range(NE):
                                        for tsub in range(2 * NBO):
                                            fps = psf.tile(
                                                [128, 512], F32, tag="fps",
                                                name=f"fps{e}_{tsub}",
                                            )
                                            for dc in range(CC):
                                                nc.tensor.matmul(
                                                    fps[:],
                                                    ot[:, dc, 128 * tsub:128 * tsub + 128],
                                                    wo[:, dc, 512 * e:512 * e + 512],
                                                    start=(dc == 0),
                                                    stop=(dc == CC - 1),
                                                )
                                            yt = stage.tile([128, 512], F32, tag="yt")
                                            nc.vector.tensor_scalar_mul(
                                                yt[:], fps[:], recipt[:, tsub:tsub + 1]
                                            )
                                            nc.sync.dma_start(
                                                y_d[128 * tsub:128 * tsub + 128, 512 * e:512 * e + 512],
                                                yt[:],
                                            )
    nc.compile()
    return nc


def _tile_w128(WT, CC):
    # [c, d] -> [dchunk, p, cc, 128] so each per-d-chunk DMA is contiguous
    return np.ascontiguousarray(
        WT.reshape(CC, 128, CC, 128).transpose(2, 1, 0, 3)
    )


def _tile_w512(WT, CC, NE):
    return np.ascontiguousarray(
        WT.reshape(CC, 128, NE, 512).transpose(2, 1, 0, 3)
    )


def _host_prep(x, Wq, Wk, Wv, Wo, T_, C_):
    CC = C_ // 128
    NE = C_ // 512
    TOWN = T_ // 2
    NBO = TOWN // 256
    ext = [4 * (bi + 1) for bi in range(NBO)]
    own_map = {0: OWN[0][:NBO], 1: OWN[1][:NBO]}
    x = np.asarray(x, np.float32)
    WqT = _tile_w128(np.asarray(Wq, np.float32).T.astype(BF16NP), CC)
    WkT = _tile_w128(np.asarray(Wk, np.float32).T.astype(BF16NP), CC)
    WvT = _tile_w512(np.asarray(Wv, np.float32).T.astype(BF16NP), CC, NE)
    WoT = _tile_w512(np.asarray(Wo, np.float32).T.astype(BF16NP), CC, NE)
    ones = np.ones((128, 2), BF16NP)
    masks = {}
    own_cols = {}
    for h in range(2):
        own = own_map[h]
        mb = np.zeros((NBO, 4, 128, 256), np.float32)
        for bi in range(NBO):
            g = own[bi]
            for rel in range(4):
                kl = ext[bi] - 4 + rel
                s_idx = 128 * kl + np.arange(128)[:, None]
                t_idx = 256 * g + np.arange(256)[None, :]
                mb[bi, rel] = np.where(s_idx <= t_idx, 0.0, NEG)
        masks[h] = mb.astype(BF16NP)
        own_cols[h] = np.concatenate(
            [np.arange(256 * g, 256 * g + 256) for g in own]
        )
    in_maps = []
    for core in range(8):
        b, h = core // 2, core % 2
        xb = x[b % x.shape[0]]
        xT = np.ascontiguousarray(xb.T).astype(BF16NP)
        xth = np.ascontiguousarray(
            xT[:, h * TOWN:(h + 1) * TOWN].reshape(CC, 128, TOWN).transpose(1, 0, 2)
        )
        xtq = np.ascontiguousarray(
            xT[:, own_cols[h]].reshape(CC, 128, TOWN).transpose(1, 0, 2)
        )
        in_maps.append(
            {
                "xth": xth,
                "xtq": xtq,
                "wq": WqT,
                "wk": WkT,
                "wv": WvT,
                "wo": WoT,
                "mb": masks[h],
                "ones": ones,
            }
        )
    return in_maps, own_cols


def kernel(x, Wq, Wk, Wv, Wo):
    from concourse.bass_utils import run_bass_kernel_spmd

    T_, C_ = T_FULL, C_FULL
    key = (T_, C_)
    if key not in _CACHE:
        _CACHE[key] = _build(T_, C_)
    nc = _CACHE[key]
    in_maps, own_cols = _host_prep(x, Wq, Wk, Wv, Wo, T_, C_)
    res = run_bass_kernel_spmd(nc, in_maps, list(range(8)))
    NBO = (T_ // 2) // 256
    y = np.zeros((B, T_, C_), np.float32)
    for core in range(8):
        b, h = core // 2, core % 2
        yc = res.results[core]["y"]
        for bi in range(NBO):
            g = OWN[h][:NBO][bi]
            y[b, 256 * g:256 * g + 256, :] = yc[256 * bi:256 * bi + 256, :]
    return y
